# revision 1
# baseline (speedup 1.0000x reference)
"""GCNConv Trainium2 kernel: 8-core SPMD via bass/Tile.

Strategy (dst-range edge sharding; one shared SPMD program, all data per-core):
  - core c owns dst nodes [c*NSH, (c+1)*NSH) and all edges into them
  - x~ = feat @ fc_w.T + edge_b table built on device. Each core's table is
    ROTATED so its own node range sits at rows [0, NSH) (keeps the program
    core-independent; the rotation is folded into the host-built gather indices)
  - per-edge: dma_gather x~[src] (512B rows; 4 src-range buckets for int16 idx),
    w = edge_feat @ edge_w.T on PE (bf16 in, fp32 acc), m = relu(dis_src*(x~+w))
    on ACT, one-hot(dst) via DVE/GPSIMD tensor_scalar, segment-sum via fp32r
    matmul into PSUM h^T [feat, nodes] super-windows at register-offset columns
  - node side: out = h*dis + relu(x+root)/deg, via PE transpose of h^T
"""
import sys, math
sys.path.insert(0, "/opt/trn_rl_repo")
import numpy as np

from concourse import bass, bacc, mybir, tile
from concourse import bass_utils

f32 = mybir.dt.float32
f32r = mybir.dt.float32r
bf16 = mybir.dt.bfloat16
i16 = mybir.dt.int16
i32 = mybir.dt.int32
RELU = mybir.ActivationFunctionType.Relu
ALU = mybir.AluOpType


class Cfg:
    def __init__(self, N=100000, E=1600000, F=128, ED=7, cores=8,
                 sw_nodes=2048, group=256, cap_full=75, cap_last=12,
                 gather_chunks=25):
        self.N, self.E, self.F, self.ED, self.cores = N, E, F, ED, cores
        assert N % cores == 0
        self.NSH = N // cores                    # nodes per core
        self.SW = sw_nodes                       # super-window width (<=4 psum banks)
        self.GRP = group                         # one-hot width / matmul N
        self.n_sw = math.ceil(self.NSH / sw_nodes)
        self.last_w = self.NSH - (self.n_sw - 1) * sw_nodes
        self.cap = [cap_full] * (self.n_sw - 1) + [cap_last]
        self.gb = gather_chunks                  # max chunks per dma_gather call
        self.n_buckets = 4
        self.bucket_sz = math.ceil(N / self.n_buckets)
        assert self.bucket_sz <= 32768
        self.n_chunks = self.n_buckets * sum(self.cap)
        self.slots = self.n_chunks * 128
        self.n_tiles = math.ceil(N / 128)        # x~ build tiles
        self.Npad = self.n_tiles * 128
        self.nsh_tiles = math.ceil(self.NSH / 128)
        self.NSHpad = self.nsh_tiles * 128

    def call_layout(self):
        """Gather-call boundaries [(slot_start, chunks_in_call)] in schedule order."""
        out = []
        si = 0
        for s in range(self.n_sw):
            for _b in range(self.n_buckets):
                rem = self.cap[s]
                while rem > 0:
                    n = min(self.gb, rem)
                    out.append((si, n))
                    si += n * 128
                    rem -= n
        assert si == self.slots
        return out


CFG = Cfg()
_PROG_CACHE = {}


# ---------------------------------------------------------------- program ----
def build_program(cfg: Cfg):
    nc = bacc.Bacc("TRN2", target_bir_lowering=False, debug=False,
                   num_devices=cfg.cores)
    F, GRP, SW = cfg.F, cfg.GRP, cfg.SW

    featT_d = nc.dram_tensor("featT", [F, cfg.Npad], f32, kind="ExternalInput")
    fcwT_d = nc.dram_tensor("fcwT", [F, F], f32, kind="ExternalInput")
    ewT8_d = nc.dram_tensor("ewT8", [8, F], bf16, kind="ExternalInput")
    edgebB_d = nc.dram_tensor("edgebB", [128, F], f32, kind="ExternalInput")
    rootB_d = nc.dram_tensor("rootB", [128, F], f32, kind="ExternalInput")
    iota_d = nc.dram_tensor("iotaG", [128, GRP], f32, kind="ExternalInput")
    ident_d = nc.dram_tensor("ident", [128, 128], f32, kind="ExternalInput")
    efT_d = nc.dram_tensor("efT", [8, cfg.slots], bf16, kind="ExternalInput")
    idx_d = nc.dram_tensor("idxw", [128, cfg.slots // 16], i16, kind="ExternalInput")
    rel_d = nc.dram_tensor("relT", [128, cfg.n_chunks], f32, kind="ExternalInput")
    disS_d = nc.dram_tensor("disS", [128, cfg.n_chunks], f32, kind="ExternalInput")
    goff_d = nc.dram_tensor("goff", [1, cfg.n_chunks], i32, kind="ExternalInput")
    disP_d = nc.dram_tensor("disP", [128, cfg.nsh_tiles], f32, kind="ExternalInput")
    ivdP_d = nc.dram_tensor("ivdP", [128, cfg.nsh_tiles], f32, kind="ExternalInput")

    xt_d = nc.dram_tensor("xtab", [cfg.Npad, F], f32, kind="Internal")
    out_d = nc.dram_tensor("out", [cfg.NSHpad, F], f32, kind="ExternalOutput")

    with tile.TileContext(nc) as tc:
        with tc.tile_pool(name="persist", bufs=1) as pers:
            fcwT = pers.tile([F, F], f32)
            nc.sync.dma_start(out=fcwT[:], in_=fcwT_d.ap())
            ewT8 = pers.tile([8, F], bf16)
            nc.sync.dma_start(out=ewT8[:], in_=ewT8_d.ap())
            edgebB = pers.tile([128, F], f32)
            nc.sync.dma_start(out=edgebB[:], in_=edgebB_d.ap())
            rootB = pers.tile([128, F], f32)
            nc.sync.dma_start(out=rootB[:], in_=rootB_d.ap())
            iotaG = pers.tile([128, GRP], f32)
            nc.sync.dma_start(out=iotaG[:], in_=iota_d.ap())
            ident = pers.tile([128, 128], f32)
            nc.sync.dma_start(out=ident[:], in_=ident_d.ap())
            zero128 = pers.tile([128, 128], bf16)
            nc.vector.memset(zero128[:], 0.0)
            zero512 = pers.tile([128, 512], bf16)
            nc.vector.memset(zero512[:], 0.0)
            relT = pers.tile([128, cfg.n_chunks], f32)
            nc.sync.dma_start(out=relT[:], in_=rel_d.ap())
            disS = pers.tile([128, cfg.n_chunks], f32)
            nc.sync.dma_start(out=disS[:], in_=disS_d.ap())
            goffT = pers.tile([1, cfg.n_chunks], i32)
            nc.sync.dma_start(out=goffT[:], in_=goff_d.ap())
            idxw = pers.tile([128, cfg.slots // 16], i16)
            nc.sync.dma_start(out=idxw[:], in_=idx_d.ap())
            disP = pers.tile([128, cfg.nsh_tiles], f32)
            nc.sync.dma_start(out=disP[:], in_=disP_d.ap())
            ivdP = pers.tile([128, cfg.nsh_tiles], f32)
            nc.sync.dma_start(out=ivdP[:], in_=ivdP_d.ap())
            hT = pers.tile([128, cfg.NSHpad], f32)   # h^T accumulator [feat, node]
            nc.vector.memset(hT[:], 0.0)

            # ================= phase 1: x~ table =================
            with (
                tc.tile_pool(name="xph", bufs=3) as xph,
                tc.tile_pool(name="xps", bufs=4, space="PSUM") as xps,
            ):
                BLK = 8
                nblk = math.ceil(cfg.n_tiles / BLK)
                for blk in range(nblk):
                    t0 = blk * BLK
                    nt = min(BLK, cfg.n_tiles - t0)
                    ft = xph.tile([F, BLK * 128], f32, tag="ft")
                    nc.sync.dma_start(
                        out=ft[:, :nt * 128],
                        in_=featT_d.ap()[:, t0 * 128:(t0 + nt) * 128])
                    xt = xph.tile([128, BLK, F], f32, tag="xt")
                    for j in range(nt):
                        px = xps.tile([128, F], f32, tag="px")
                        nc.tensor.matmul(out=px[:], lhsT=ft[:, j * 128:(j + 1) * 128],
                                         rhs=fcwT[:], start=True, stop=True)
                        nc.vector.tensor_add(out=xt[:, j, :], in0=px[:], in1=edgebB[:])
                    nc.sync.dma_start(
                        out=xt_d.ap()[t0 * 128:(t0 + nt) * 128, :].rearrange(
                            "(b p) f -> p b f", p=128),
                        in_=xt[:, :nt, :])

            # ================= phase 2: edges =================
            with (
                tc.tile_pool(name="eph", bufs=2) as eph,
                tc.tile_pool(name="mph", bufs=4) as mph,
                tc.tile_pool(name="hps_pool", bufs=1, space="PSUM") as hps_pool,
                tc.tile_pool(name="wps_pool", bufs=4, space="PSUM") as wps_pool,
            ):
                hps = hps_pool.tile([128, SW], f32)
                ci = 0
                si = 0
                for sw in range(cfg.n_sw):
                    for bank in range(SW // 512):
                        nc.tensor.matmul(
                            out=hps[:, bank * 512:(bank + 1) * 512],
                            lhsT=zero128[:],
                            rhs=zero512[:],
                            start=True, stop=False, skip_group_check=True)
                    for b in range(cfg.n_buckets):
                        base = b * cfg.bucket_sz
                        bucket_ap = xt_d.ap()[base:min(base + cfg.bucket_sz,
                                                       cfg.Npad), :]
                        rem = cfg.cap[sw]
                        call_sizes = []
                        while rem > 0:
                            call_sizes.append(min(cfg.gb, rem))
                            rem -= call_sizes[-1]
                        for ncall in call_sizes:
                            nidx = ncall * 128
                            gout = eph.tile([128, cfg.gb, F], f32, tag="gout")
                            nc.gpsimd.dma_gather(
                                out_ap=gout[:, :ncall, :],
                                in_ap=bucket_ap,
                                idxs_ap=idxw[:, si // 16:(si + nidx) // 16],
                                num_idxs=nidx, num_idxs_reg=nidx, elem_size=F,
                                single_packet=False)
                            ef = eph.tile([8, cfg.gb * 128], bf16, tag="ef")
                            nc.sync.dma_start(
                                out=ef[:, :nidx], in_=efT_d.ap()[:, si:si + nidx])
                            for kk in range(ncall):
                                pw = wps_pool.tile([128, F], f32, tag="pw")
                                nc.tensor.matmul(
                                    out=pw[:], lhsT=ef[:, kk * 128:(kk + 1) * 128],
                                    rhs=ewT8[:], start=True, stop=True)
                                mpre = mph.tile([128, F], f32, tag="mpre")
                                nc.vector.tensor_add(
                                    out=mpre[:], in0=gout[:, kk, :], in1=pw[:])
                                m = mph.tile([128, F], f32r, tag="m")
                                nc.scalar.activation(
                                    out=m[:], in_=mpre[:], func=RELU,
                                    scale=disS[:, ci:ci + 1])
                                oh = mph.tile([128, GRP], f32r, tag="oh")
                                eng = nc.vector if (ci % 2 == 0) else nc.gpsimd
                                eng.tensor_scalar(
                                    out=oh[:], in0=iotaG[:],
                                    scalar1=relT[:, ci:ci + 1], op0=ALU.subtract,
                                    scalar2=0.0, op1=ALU.is_equal)
                                with tc.tile_critical():
                                    reg = nc.tensor.alloc_register(f"go{ci}")
                                    nc.tensor.reg_load(reg, goffT[0:1, ci:ci + 1])
                                    val = nc.snap(reg, donate=True, min_val=0,
                                                  max_val=SW - GRP)
                                    nc.tensor.matmul(
                                        out=hps[:, bass.ds(val, GRP)],
                                        lhsT=m[:],
                                        rhs=oh[:],
                                        start=False, stop=False,
                                        skip_group_check=True)
                                ci += 1
                                si += 128
                    w = SW if sw < cfg.n_sw - 1 else cfg.last_w
                    nc.vector.tensor_add(
                        out=hT[:, sw * SW:sw * SW + w],
                        in0=hT[:, sw * SW:sw * SW + w], in1=hps[:, :w])
                assert ci == cfg.n_chunks and si == cfg.slots

            # ================= phase 3: node-side =================
            with (
                tc.tile_pool(name="nph", bufs=3) as nph,
                tc.tile_pool(name="nps", bufs=4, space="PSUM") as nps,
            ):
                NBLK = 8
                for blk in range(math.ceil(cfg.nsh_tiles / NBLK)):
                    t0 = blk * NBLK
                    nt = min(NBLK, cfg.nsh_tiles - t0)
                    xtile = nph.tile([128, NBLK, F], f32, tag="xtile")
                    nc.sync.dma_start(
                        out=xtile[:, :nt, :],
                        in_=xt_d.ap()[t0 * 128:(t0 + nt) * 128, :].rearrange(
                            "(b p) f -> p b f", p=128))
                    ot = nph.tile([128, NBLK, F], f32, tag="ot")
                    for j in range(nt):
                        t = t0 + j
                        pt = nps.tile([128, F], f32, tag="pt")
                        nc.tensor.transpose(
                            out=pt[:], in_=hT[:, t * 128:(t + 1) * 128],
                            identity=ident[:])
                        s1 = nph.tile([128, F], f32, tag="s1")
                        nc.vector.tensor_scalar_mul(
                            out=s1[:], in0=pt[:], scalar1=disP[:, t:t + 1])
                        t1 = nph.tile([128, F], f32, tag="t1")
                        nc.vector.tensor_add(
                            out=t1[:], in0=xtile[:, j, :], in1=rootB[:])
                        s2 = nph.tile([128, F], f32, tag="s2")
                        nc.scalar.activation(
                            out=s2[:], in_=t1[:], func=RELU,
                            scale=ivdP[:, t:t + 1])
                        nc.vector.tensor_add(out=ot[:, j, :], in0=s1[:], in1=s2[:])
                    nc.sync.dma_start(
                        out=out_d.ap()[t0 * 128:(t0 + nt) * 128, :].rearrange(
                            "(b p) f -> p b f", p=128),
                        in_=ot[:, :nt, :])
    nc.compile()
    return nc


# ------------------------------------------------------------- host prep ----
def host_prep(cfg: Cfg, feat, edge_feat, src, dst, fc_w, edge_w, edge_b,
              root_emb):
    N, E, F = cfg.N, cfg.E, cfg.F
    deg = (np.bincount(dst, minlength=N) + 1.0).astype(np.float32)
    dis = deg ** -0.5

    featT_full = np.ascontiguousarray(feat.T).astype(np.float32)   # [F, N]
    fcwT = np.ascontiguousarray(fc_w.T).astype(np.float32)
    ewT8 = np.zeros((8, F), dtype=np.float32)
    ewT8[:cfg.ED] = edge_w.T
    edgebB = np.tile(edge_b[None, :], (128, 1)).astype(np.float32)
    rootB = np.tile((root_emb[0] - edge_b)[None, :], (128, 1)).astype(np.float32)
    iotaG = np.tile(np.arange(cfg.GRP, dtype=np.float32), (128, 1))
    ident = np.eye(128, dtype=np.float32)

    core_of = dst // cfg.NSH
    in_maps = []
    for c in range(cfg.cores):
        sel = np.nonzero(core_of == c)[0]
        # rotated node space: node v -> row (v - c*NSH) mod N
        rsrc = (src[sel] - c * cfg.NSH) % N
        ed = dst[sel] - c * cfg.NSH
        eb = rsrc // cfg.bucket_sz
        sw = ed // cfg.SW
        order = np.lexsort((ed, eb, sw))
        es, ed, eb, sw = rsrc[order], ed[order], eb[order], sw[order]
        eid = sel[order]

        slot_src = np.zeros(cfg.slots, dtype=np.int16)
        slot_rel = np.full(cfg.slots, -1.0, dtype=np.float32)
        slot_dis = np.zeros(cfg.slots, dtype=np.float32)
        slot_eid = np.full(cfg.slots, -1, dtype=np.int64)
        chunk_goff = np.zeros(cfg.n_chunks, dtype=np.int32)

        ci = 0
        comp = sw * cfg.n_buckets + eb
        seg_starts = np.searchsorted(comp, np.arange(cfg.n_sw * cfg.n_buckets + 1))
        for s in range(cfg.n_sw):
            for b in range(cfg.n_buckets):
                lo = seg_starts[s * cfg.n_buckets + b]
                hi = seg_starts[s * cfg.n_buckets + b + 1]
                seg_ed, seg_es, seg_eid = ed[lo:hi], es[lo:hi], eid[lo:hi]
                grp = (seg_ed - s * cfg.SW) // cfg.GRP
                cstart = ci
                gi = 0
                nseg = len(seg_ed)
                while gi < nseg:
                    gj = gi
                    g = int(grp[gi])
                    while gj < nseg and grp[gj] == g:
                        gj += 1
                    for a in range(gi, gj, 128):
                        z = min(a + 128, gj)
                        slot0 = ci * 128
                        n = z - a
                        srcs = seg_es[a:z]
                        slot_src[slot0:slot0 + n] = (
                            srcs - b * cfg.bucket_sz).astype(np.int16)
                        slot_rel[slot0:slot0 + n] = (
                            seg_ed[a:z] - s * cfg.SW - g * cfg.GRP)
                        slot_dis[slot0:slot0 + n] = dis[src[seg_eid[a:z]]]
                        slot_eid[slot0:slot0 + n] = seg_eid[a:z]
                        chunk_goff[ci] = g * cfg.GRP
                        ci += 1
                    gi = gj
                used = ci - cstart
                if used > cfg.cap[s]:
                    raise RuntimeError(
                        f"segment overflow core {c} sw {s} bucket {b}: "
                        f"{used} > {cfg.cap[s]}")
                ci = cstart + cfg.cap[s]
        assert ci == cfg.n_chunks

        real = slot_eid >= 0
        efT = np.zeros((8, cfg.slots), dtype=np.float32)
        efT[:cfg.ED, real] = edge_feat[slot_eid[real]].T
        idxw = np.zeros((16, cfg.slots // 16), dtype=np.int16)
        for s0, nch in cfg.call_layout():
            blkv = slot_src[s0:s0 + nch * 128]
            idxw[:, s0 // 16:(s0 + nch * 128) // 16] = blkv.reshape(-1, 16).T
        idxw = np.tile(idxw, (8, 1))
        relT = np.ascontiguousarray(slot_rel.reshape(-1, 128).T)
        disS = np.ascontiguousarray(slot_dis.reshape(-1, 128).T)

        nd = np.arange(cfg.NSHpad)
        gidx = np.minimum(c * cfg.NSH + nd, N - 1)
        disP = np.ascontiguousarray(dis[gidx].reshape(-1, 128).T)
        ivdP = np.ascontiguousarray((1.0 / deg[gidx]).reshape(-1, 128).T)

        featT = np.zeros((F, cfg.Npad), dtype=np.float32)
        featT[:, :N] = np.roll(featT_full, -c * cfg.NSH, axis=1)

        in_maps.append({
            "featT": featT, "fcwT": fcwT, "ewT8": ewT8,
            "edgebB": edgebB, "rootB": rootB, "iotaG": iotaG, "ident": ident,
            "efT": efT, "idxw": idxw, "relT": relT, "disS": disS,
            "goff": np.ascontiguousarray(chunk_goff.reshape(1, -1)),
            "disP": disP, "ivdP": ivdP,
        })
    return in_maps


def _cast_maps(in_maps):
    import ml_dtypes
    for m in in_maps:
        m["ewT8"] = m["ewT8"].astype(ml_dtypes.bfloat16)
        m["efT"] = m["efT"].astype(ml_dtypes.bfloat16)
    return in_maps


# ----------------------------------------------------------------- entry ----
def kernel(feat, edge_feat, src, dst, fc_w, edge_w, edge_b, root_emb,
           _trace=False, _cfg=None, **_kw):
    cfg = _cfg or CFG
    feat = np.asarray(feat); edge_feat = np.asarray(edge_feat)
    src = np.asarray(src); dst = np.asarray(dst)
    fc_w = np.asarray(fc_w); edge_w = np.asarray(edge_w)
    edge_b = np.asarray(edge_b); root_emb = np.asarray(root_emb)
    assert feat.shape == (cfg.N, cfg.F) and src.shape == (cfg.E,), \
        (feat.shape, src.shape)
    key = id(cfg) if _cfg is not None else "main"
    if key not in _PROG_CACHE:
        _PROG_CACHE[key] = build_program(cfg)
    nc = _PROG_CACHE[key]
    in_maps = _cast_maps(host_prep(
        cfg, feat, edge_feat, src, dst, fc_w, edge_w, edge_b, root_emb))
    res = bass_utils.run_bass_kernel_spmd(
        nc, in_maps, core_ids=list(range(cfg.cores)), trace=_trace)
    out = np.concatenate(
        [res.results[c]["out"][:cfg.NSH] for c in range(cfg.cores)], axis=0)
    kernel._last_results = res
    return out.astype(np.float32)



# revision 10
# speedup vs baseline: 3.3926x; 3.3926x over previous
"""GCNConv Trainium2 kernel: 8-core SPMD via bass/Tile.

Strategy (dst-range edge sharding; one shared SPMD program, all data per-core):
  - core c owns dst nodes [c*NSH, (c+1)*NSH) and all edges into them
  - x~ = feat @ fc_w.T + edge_b table (bf16) built on device. Each core's
    table is ROTATED so its own node range sits at rows [0, NSH)
  - STATIC edge schedule: edges bucketed by (dst super-window s, src bucket b,
    dst group g); per-(s,b,g) chunk capacities are computed from the actual
    input (max over the 8 cores) and baked into the program, so every PSUM
    column offset is a compile-time constant — no registers, no critical
    sections
  - per-edge: dma_gather x~[src] (256B bf16 rows, gather calls round-robined
    over SWDGE queues), w = edge_feat @ edge_w.T on PE, m = relu(x~+w) on ACT,
    weighted one-hot (is_equal * dis_src*dis_dst) on DVE, segment-sum via bf16
    matmul into PSUM h^T window at static column offset
  - node side inlined per super-window: h^T -> transpose -> + relu(x+root)/deg
"""
import sys, math
sys.path.insert(0, "/opt/trn_rl_repo")
import numpy as np

from concourse import bass, bacc, mybir, tile
from concourse import bass_utils

f32 = mybir.dt.float32
bf16 = mybir.dt.bfloat16
i16 = mybir.dt.int16
RELU = mybir.ActivationFunctionType.Relu
ALU = mybir.AluOpType


class Cfg:
    def __init__(self, caps, N=100000, E=1600000, F=128, ED=7, cores=8,
                 sw_nodes=2048, group=256, gather_chunks=48, n_queues=1):
        self.N, self.E, self.F, self.ED, self.cores = N, E, F, ED, cores
        assert N % cores == 0
        self.NSH = N // cores
        self.SW = sw_nodes
        self.GRP = group
        self.n_sw = math.ceil(self.NSH / sw_nodes)
        self.gb = gather_chunks
        self.nq = n_queues
        self.n_buckets = 4
        self.bucket_sz = math.ceil(N / self.n_buckets)
        assert self.bucket_sz <= 32768
        self.n_tiles = math.ceil(N / 128)
        self.Npad = self.n_tiles * 128
        self.nsh_tiles = math.ceil(self.NSH / 128)
        self.NSHpad = self.nsh_tiles * 128
        # caps[s][b][g]: chunk capacity per (super-window, src bucket, group)
        self.caps = caps
        self.n_chunks = sum(sum(sum(bg) for bg in sb) for sb in caps)
        self.slots = self.n_chunks * 128

    def win_w(self, s):
        return min(self.SW, self.NSH - s * self.SW)

    def n_groups(self, s):
        return math.ceil(self.win_w(s) / self.GRP)

    def sw_tiles(self, s):
        return math.ceil(self.win_w(s) / 128)

    def static_walk(self):
        """Chunk records in schedule order: (s, b, g, start_flag)."""
        out = []
        for s in range(self.n_sw):
            seen = set()
            for b in range(self.n_buckets):
                for g in range(self.n_groups(s)):
                    for _ in range(self.caps[s][b][g]):
                        out.append((s, b, g, g not in seen))
                        seen.add(g)
        return out

    def call_layout(self):
        """Gather calls: (s, b, chunk_start_index, n_chunks_in_call)."""
        out = []
        ci = 0
        for s in range(self.n_sw):
            for b in range(self.n_buckets):
                rem = sum(self.caps[s][b][g] for g in range(self.n_groups(s)))
                while rem > 0:
                    n = min(self.gb, rem)
                    out.append((s, b, ci, n))
                    ci += n
                    rem -= n
        assert ci == self.n_chunks
        return out


_PROG_CACHE = {}


def compute_caps(cfg_shape, src, dst):
    """caps[s][b][g] = max over cores of ceil(edge-count/128), >=1 coverage."""
    N, NSH, SW, GRP = (cfg_shape['N'], cfg_shape['NSH'], cfg_shape['SW'],
                       cfg_shape['GRP'])
    n_buckets = cfg_shape['n_buckets']
    bucket_sz = cfg_shape['bucket_sz']
    cores = cfg_shape['cores']
    n_sw = math.ceil(NSH / SW)
    gmax = SW // GRP
    core_of = dst // NSH
    counts = np.zeros((cores, n_sw, n_buckets, gmax), dtype=np.int64)
    for c in range(cores):
        sel = core_of == c
        rsrc = (src[sel] - c * NSH) % N
        ed = dst[sel] - c * NSH
        b = rsrc // bucket_sz
        s = ed // SW
        g = (ed - s * SW) // GRP
        np.add.at(counts[c], (s, b, g), 1)
    need = -(-counts // 128)            # ceil
    cap = need.max(axis=0)              # [n_sw, n_buckets, gmax]
    caps = []
    for s in range(n_sw):
        ng = math.ceil(min(SW, NSH - s * SW) / GRP)
        sb = []
        for b in range(n_buckets):
            sb.append([int(cap[s, b, g]) for g in range(ng)])
        caps.append(sb)
    return caps


# ---------------------------------------------------------------- program ----
def build_program(cfg: Cfg):
    nc = bacc.Bacc("TRN2", target_bir_lowering=False, debug=False,
                   num_devices=cfg.cores)
    F, GRP, SW = cfg.F, cfg.GRP, cfg.SW

    featT_d = nc.dram_tensor("featT", [F, cfg.Npad], bf16, kind="ExternalInput")
    fcwT_d = nc.dram_tensor("fcwT", [F, F], bf16, kind="ExternalInput")
    ewT8_d = nc.dram_tensor("ewT8", [8, F], bf16, kind="ExternalInput")
    edgebB_d = nc.dram_tensor("edgebB", [128, F], f32, kind="ExternalInput")
    rootB_d = nc.dram_tensor("rootB", [128, F], f32, kind="ExternalInput")
    iota_d = nc.dram_tensor("iotaG", [128, GRP], bf16, kind="ExternalInput")
    ident_d = nc.dram_tensor("ident", [128, 128], bf16, kind="ExternalInput")
    efT_d = nc.dram_tensor("efT", [8, cfg.slots], bf16, kind="ExternalInput")
    idx_d = nc.dram_tensor("idxw", [128, cfg.slots // 16], i16,
                           kind="ExternalInput")
    rel_d = nc.dram_tensor("relT", [128, cfg.n_chunks], f32,
                           kind="ExternalInput")
    disS_d = nc.dram_tensor("disS", [128, cfg.n_chunks], f32,
                            kind="ExternalInput")
    ivdP_d = nc.dram_tensor("ivdP", [128, cfg.nsh_tiles], f32,
                            kind="ExternalInput")

    xt_d = nc.dram_tensor("xtab", [cfg.Npad, F], bf16, kind="Internal")
    out_d = nc.dram_tensor("out", [cfg.NSHpad, F], f32, kind="ExternalOutput")

    walk = cfg.static_walk()
    calls = cfg.call_layout()

    with tile.TileContext(nc) as tc:
        with tc.tile_pool(name="persist", bufs=1) as pers:
            fcwT = pers.tile([F, F], bf16)
            nc.sync.dma_start(out=fcwT[:], in_=fcwT_d.ap())
            ewT8 = pers.tile([8, F], bf16)
            nc.sync.dma_start(out=ewT8[:], in_=ewT8_d.ap())
            edgebB = pers.tile([128, F], f32)
            nc.sync.dma_start(out=edgebB[:], in_=edgebB_d.ap())
            rootB = pers.tile([128, F], f32)
            nc.sync.dma_start(out=rootB[:], in_=rootB_d.ap())
            iotaG = pers.tile([128, GRP], bf16)
            nc.sync.dma_start(out=iotaG[:], in_=iota_d.ap())
            ident = pers.tile([128, 128], bf16)
            nc.sync.dma_start(out=ident[:], in_=ident_d.ap())
            relT = pers.tile([128, cfg.n_chunks], f32)
            nc.sync.dma_start(out=relT[:], in_=rel_d.ap())
            disS = pers.tile([128, cfg.n_chunks], f32)
            nc.sync.dma_start(out=disS[:], in_=disS_d.ap())
            idxw = pers.tile([128, cfg.slots // 16], i16)
            nc.sync.dma_start(out=idxw[:], in_=idx_d.ap())
            ivdP = pers.tile([128, cfg.nsh_tiles], f32)
            nc.sync.dma_start(out=ivdP[:], in_=ivdP_d.ap())
            zero128 = pers.tile([128, 128], bf16)
            nc.vector.memset(zero128[:], 0.0)
            zero512 = pers.tile([128, 512], bf16)
            nc.vector.memset(zero512[:], 0.0)

            # ================= phase 1: x~ table (bf16) =================
            with (
                tc.tile_pool(name="xph", bufs=3) as xph,
                tc.tile_pool(name="xps", bufs=4, space="PSUM") as xps,
            ):
                BLK = 8
                nblk = math.ceil(cfg.n_tiles / BLK)
                for blk in range(nblk):
                    t0 = blk * BLK
                    nt = min(BLK, cfg.n_tiles - t0)
                    ft = xph.tile([F, BLK * 128], bf16, tag="ft")
                    nc.sync.dma_start(
                        out=ft[:, :nt * 128],
                        in_=featT_d.ap()[:, t0 * 128:(t0 + nt) * 128])
                    xt = xph.tile([128, BLK, F], bf16, tag="xt")
                    for j in range(nt):
                        px = xps.tile([128, F], f32, tag="px")
                        nc.tensor.matmul(out=px[:], lhsT=ft[:, j * 128:(j + 1) * 128],
                                         rhs=fcwT[:], start=True, stop=True)
                        nc.vector.tensor_add(out=xt[:, j, :], in0=px[:], in1=edgebB[:])
                    nc.sync.dma_start(
                        out=xt_d.ap()[t0 * 128:(t0 + nt) * 128, :].rearrange(
                            "(b p) f -> p b f", p=128),
                        in_=xt[:, :nt, :])

            # ============ phase 2+3: edges + node-side per super-window ======
            with (
                tc.tile_pool(name="eph", bufs=2) as eph,
                tc.tile_pool(name="mph", bufs=4) as mph,
                tc.tile_pool(name="hps_pool", bufs=1, space="PSUM") as hps_pool,
                tc.tile_pool(name="wps_pool", bufs=2, space="PSUM") as wps_pool,
                tc.tile_pool(name="nps", bufs=2, space="PSUM") as nps,
                tc.tile_pool(name="nph", bufs=2) as nph,
                tc.tile_pool(name="hsp", bufs=2) as hsp,
            ):
                hps = hps_pool.tile([128, SW], f32)
                ci = 0          # global chunk index
                call_i = 0
                for s in range(cfg.n_sw):
                    # zero the PSUM banks of this super-window (matmul
                    # start=True resets the WHOLE 2KB bank, so per-group
                    # first-touch init would wipe the sibling group)
                    for bank in range(SW // 512):
                        nc.tensor.matmul(
                            out=hps[:, bank * 512:(bank + 1) * 512],
                            lhsT=zero128[:], rhs=zero512[:],
                            start=True, stop=False, skip_group_check=True)
                    # ---- edge chunks of this super-window ----
                    while call_i < len(calls) and calls[call_i][0] == s:
                        _, b, c0, ncall = calls[call_i]
                        nidx = ncall * 128
                        si = c0 * 128
                        base = b * cfg.bucket_sz
                        bucket_ap = xt_d.ap()[base:min(base + cfg.bucket_sz,
                                                       cfg.Npad), :]
                        gout = eph.tile([128, cfg.gb, F], bf16, tag="gout")
                        nc.gpsimd.dma_gather(
                            out_ap=gout[:, :ncall, :],
                            in_ap=bucket_ap,
                            idxs_ap=idxw[:, si // 16:(si + nidx) // 16],
                            num_idxs=nidx, num_idxs_reg=nidx, elem_size=F,
                            single_packet=False,
                            queue_num=call_i % cfg.nq)
                        ef = eph.tile([8, cfg.gb * 128], bf16, tag="ef")
                        nc.sync.dma_start(
                            out=ef[:, :nidx], in_=efT_d.ap()[:, si:si + nidx])
                        for kk in range(ncall):
                            _s, _b, g, _start = walk[ci]
                            pw = wps_pool.tile([128, F], f32, tag="pw")
                            nc.tensor.matmul(
                                out=pw[:], lhsT=ef[:, kk * 128:(kk + 1) * 128],
                                rhs=ewT8[:], start=True, stop=True)
                            mpre = mph.tile([128, F], bf16, tag="mpre")
                            nc.vector.tensor_add(
                                out=mpre[:], in0=gout[:, kk, :], in1=pw[:])
                            m = mph.tile([128, F], bf16, tag="m")
                            nc.scalar.activation(out=m[:], in_=mpre[:], func=RELU)
                            oh = mph.tile([128, GRP], bf16, tag="oh")
                            nc.vector.tensor_scalar(
                                out=oh[:], in0=iotaG[:],
                                scalar1=relT[:, ci:ci + 1], op0=ALU.is_equal,
                                scalar2=disS[:, ci:ci + 1], op1=ALU.mult)
                            nc.tensor.matmul(
                                out=hps[:, g * GRP:(g + 1) * GRP],
                                lhsT=m[:], rhs=oh[:],
                                start=False, stop=False,
                                skip_group_check=True)
                            ci += 1
                        call_i += 1
                    # ---- node-side for this super-window ----
                    ntp = cfg.sw_tiles(s)
                    hstage = hsp.tile([128, SW], bf16, tag="hstage")
                    nc.scalar.copy(out=hstage[:, :ntp * 128],
                                   in_=hps[:, :ntp * 128])
                    xtile = nph.tile([128, 16, F], bf16, tag="xt3")
                    r0 = s * SW
                    nc.sync.dma_start(
                        out=xtile[:, :ntp, :],
                        in_=xt_d.ap()[r0:r0 + ntp * 128, :].rearrange(
                            "(b p) f -> p b f", p=128))
                    ot = nph.tile([128, 16, F], f32, tag="ot")
                    for j in range(ntp):
                        t = s * (SW // 128) + j
                        pt = nps.tile([128, F], bf16, tag="pt")
                        nc.tensor.transpose(
                            out=pt[:], in_=hstage[:, j * 128:(j + 1) * 128],
                            identity=ident[:])
                        t1 = nph.tile([128, F], f32, tag="t1")
                        nc.vector.tensor_add(
                            out=t1[:], in0=xtile[:, j, :], in1=rootB[:])
                        s2 = nph.tile([128, F], f32, tag="s2")
                        nc.scalar.activation(
                            out=s2[:], in_=t1[:], func=RELU,
                            scale=ivdP[:, t:t + 1])
                        nc.vector.tensor_add(out=ot[:, j, :], in0=pt[:], in1=s2[:])
                    nc.sync.dma_start(
                        out=out_d.ap()[r0:r0 + ntp * 128, :].rearrange(
                            "(b p) f -> p b f", p=128),
                        in_=ot[:, :ntp, :])
                assert ci == cfg.n_chunks
    nc.compile()
    return nc


# ------------------------------------------------------------- host prep ----
def host_prep(cfg: Cfg, feat, edge_feat, src, dst, fc_w, edge_w, edge_b,
              root_emb):
    import ml_dtypes
    N, E, F = cfg.N, cfg.E, cfg.F
    deg = (np.bincount(dst, minlength=N) + 1.0).astype(np.float32)
    dis = deg ** -0.5

    featT_full = np.ascontiguousarray(feat.T).astype(np.float32)   # [F, N]
    fcwT = np.ascontiguousarray(fc_w.T).astype(ml_dtypes.bfloat16)
    ewT8 = np.zeros((8, F), dtype=np.float32)
    ewT8[:cfg.ED] = edge_w.T
    ewT8 = ewT8.astype(ml_dtypes.bfloat16)
    edgebB = np.tile(edge_b[None, :], (128, 1)).astype(np.float32)
    rootB = np.tile((root_emb[0] - edge_b)[None, :], (128, 1)).astype(np.float32)
    iotaG = np.tile(np.arange(cfg.GRP, dtype=np.float32),
                    (128, 1)).astype(ml_dtypes.bfloat16)
    ident = np.eye(128, dtype=np.float32).astype(ml_dtypes.bfloat16)

    walk = cfg.static_walk()
    calls = cfg.call_layout()
    # chunk index ranges per (s,b,g) segment in schedule order
    seg_of = {}
    ci = 0
    for s in range(cfg.n_sw):
        for b in range(cfg.n_buckets):
            for g in range(cfg.n_groups(s)):
                cp = cfg.caps[s][b][g]
                seg_of[(s, b, g)] = (ci, cp)
                ci += cp
    assert ci == cfg.n_chunks

    core_of = dst // cfg.NSH
    in_maps = []
    for c in range(cfg.cores):
        sel = np.nonzero(core_of == c)[0]
        rsrc = (src[sel] - c * cfg.NSH) % N
        ed = dst[sel] - c * cfg.NSH
        eb = rsrc // cfg.bucket_sz
        sw = ed // cfg.SW
        gg = (ed - sw * cfg.SW) // cfg.GRP
        order = np.lexsort((gg, eb, sw))
        es, ed2, eb2, sw2, gg2 = (rsrc[order], ed[order], eb[order],
                                  sw[order], gg[order])
        eid = sel[order]

        slot_src = np.zeros(cfg.slots, dtype=np.int16)
        slot_rel = np.full(cfg.slots, -1.0, dtype=np.float32)
        slot_dis = np.zeros(cfg.slots, dtype=np.float32)
        slot_eid = np.full(cfg.slots, -1, dtype=np.int64)

        gmax = cfg.SW // cfg.GRP
        comp = (sw2 * cfg.n_buckets + eb2) * gmax + gg2
        seg_starts = np.searchsorted(
            comp, np.arange(cfg.n_sw * cfg.n_buckets * gmax + 1))
        for s in range(cfg.n_sw):
            for b in range(cfg.n_buckets):
                for g in range(cfg.n_groups(s)):
                    k = (s * cfg.n_buckets + b) * gmax + g
                    lo, hi = seg_starts[k], seg_starts[k + 1]
                    nseg = hi - lo
                    c0, cp = seg_of[(s, b, g)]
                    if nseg > cp * 128:
                        raise RuntimeError(
                            f"segment overflow core {c} s{s} b{b} g{g}: "
                            f"{nseg} > {cp * 128}")
                    if nseg == 0:
                        continue
                    slot0 = c0 * 128
                    slot_src[slot0:slot0 + nseg] = (
                        es[lo:hi] - b * cfg.bucket_sz).astype(np.int16)
                    slot_rel[slot0:slot0 + nseg] = (
                        ed2[lo:hi] - s * cfg.SW - g * cfg.GRP)
                    seg_eid = eid[lo:hi]
                    slot_dis[slot0:slot0 + nseg] = (
                        dis[src[seg_eid]] * dis[dst[seg_eid]])
                    slot_eid[slot0:slot0 + nseg] = seg_eid

        real = slot_eid >= 0
        efT = np.zeros((8, cfg.slots), dtype=np.float32)
        efT[:cfg.ED, real] = edge_feat[slot_eid[real]].T
        efT = efT.astype(ml_dtypes.bfloat16)
        idxw = np.zeros((16, cfg.slots // 16), dtype=np.int16)
        for _s, _b, c0, nch in calls:
            s0 = c0 * 128
            blkv = slot_src[s0:s0 + nch * 128]
            idxw[:, s0 // 16:(s0 + nch * 128) // 16] = blkv.reshape(-1, 16).T
        idxw = np.tile(idxw, (8, 1))
        relT = np.ascontiguousarray(slot_rel.reshape(-1, 128).T)
        disSv = np.ascontiguousarray(slot_dis.reshape(-1, 128).T)

        nd = np.arange(cfg.NSHpad)
        gidx = np.minimum(c * cfg.NSH + nd, N - 1)
        ivdP = np.ascontiguousarray((1.0 / deg[gidx]).reshape(-1, 128).T)

        featT = np.zeros((F, cfg.Npad), dtype=np.float32)
        featT[:, :N] = np.roll(featT_full, -c * cfg.NSH, axis=1)
        featT = featT.astype(ml_dtypes.bfloat16)

        in_maps.append({
            "featT": featT, "fcwT": fcwT, "ewT8": ewT8,
            "edgebB": edgebB, "rootB": rootB, "iotaG": iotaG, "ident": ident,
            "efT": efT, "idxw": idxw, "relT": relT, "disS": disSv,
            "ivdP": ivdP,
        })
    return in_maps


# ----------------------------------------------------------------- entry ----
def kernel(feat, edge_feat, src, dst, fc_w, edge_w, edge_b, root_emb,
           _trace=False, **_kw):
    feat = np.asarray(feat); edge_feat = np.asarray(edge_feat)
    src = np.asarray(src); dst = np.asarray(dst)
    fc_w = np.asarray(fc_w); edge_w = np.asarray(edge_w)
    edge_b = np.asarray(edge_b); root_emb = np.asarray(root_emb)

    shape = dict(N=100000, NSH=12500, SW=2048, GRP=256, n_buckets=4,
                 bucket_sz=25000, cores=8)
    caps = compute_caps(shape, src, dst)
    cfg = Cfg(caps)
    assert feat.shape == (cfg.N, cfg.F) and src.shape == (cfg.E,), \
        (feat.shape, src.shape)
    key = str(caps)
    if key not in _PROG_CACHE:
        _PROG_CACHE[key] = build_program(cfg)
    nc = _PROG_CACHE[key]
    in_maps = host_prep(
        cfg, feat, edge_feat, src, dst, fc_w, edge_w, edge_b, root_emb)
    res = bass_utils.run_bass_kernel_spmd(
        nc, in_maps, core_ids=list(range(cfg.cores)), trace=_trace)
    out = np.concatenate(
        [res.results[c]["out"][:cfg.NSH] for c in range(cfg.cores)], axis=0)
    kernel._last_results = res
    return out.astype(np.float32)
